# revision 41
# baseline (speedup 1.0000x reference)
"""Trainium2 Bass kernel for masked bi-linear attention.

Computes, for full inputs
    k:    [B, KL, E] f32
    q:    [B, Q,  E] f32
    W:    [E, E]     f32
    mask: [B, Q, KL] i32 (0/1)
the reference
    qw    = q @ W                      [B, Q, E]
    s     = qw @ k^T                   [B, Q, KL]
    p     = softmax(s, axis=-1) * mask
    out   = p @ k                      [B, Q, E]

Sharding: data-parallel over B across 8 NeuronCores (2 batches/core),
W replicated. Each core runs the same Bass program on its B-slice.

Precision strategy (grading gate is rel_err < 2e-2; scores have std ~32
so softmax is peaked and score precision matters most):
  - qw and s matmuls in float32r (PE reads fp32, truncates to fp22) at
    full bf16 rate -> score abs err ~2e-3, output rel err ~1e-3.
  - softmax (max/exp/sum) in fp32 on DVE/ACT; exp reads score PSUM
    directly (per-KB-block partial maxes, fused PSUM->SBUF exp).
  - p and the PV matmul in bf16 (adds ~2e-3 output rel err; p transposes
    then get bf16 fast-weight-load, halving their PE cost).

Pipelining: per-q-tile PV work is deferred by one tile (two at block
edges, so the older tile's PV fills the PE while the next block's q
DMAs/transpose copies land); p-transposes run one group ahead of the PV
matmuls; k chunks live in a 16-slot pool so the next batch's k DMA
pipelines behind the previous batch's last PV; the per-batch k phase is
split around qw to cover the W/k DMA streams.

Measured on trn2 (8 cores, axon), best of repeated runs: ~776-790 us,
rel err 2.09e-3 (gate 2e-2). Note ~half of runs see the PE clock at
2.0 GHz instead of 2.4 (chip power throttle) and measure ~1.2x slower;
judge changes by best-of-3. Previous 3-pass-bf16 baseline: 1522 us.
"""

import numpy as np

import concourse.bacc as bacc
import concourse.mybir as mybir
import concourse.tile as tile
from concourse.bass_utils import run_bass_kernel_spmd
from concourse.masks import make_identity
from contextlib import ExitStack

dt = mybir.dt
AF = mybir.ActivationFunctionType
ALU = mybir.AluOpType
AX = mybir.AxisListType

P = 128

N_CORES = 8
B, Q_LEN, K_LEN, EMB = 16, 2048, 2048, 1024


def emit_attention(ctx, tc, k_ap, q_ap, w_ap, mask_ap, out_ap,
                   Bl, Q, KL, E, QB=512):
    nc = tc.nc
    f32, bf16, i32, f32r = dt.float32, dt.bfloat16, dt.int32, dt.float32r

    assert Q % QB == 0 and QB % P == 0 and KL % P == 0 and E % P == 0
    EC = E // P          # e (contraction for qw) chunks        = 8
    KC = KL // P         # k chunks                             = 16
    FC = E // P          # f chunks (qw output tiles)           = 8
    nqb = Q // QB        # q blocks per batch                   = 4
    qt_per_b = QB // P   # q tiles per block                    = 4
    KB = min(512, KL)    # score psum block (1 bank)
    nkb = KL // KB       # = 4
    EB = min(512, E)     # PV psum block
    neb = E // EB        # = 2
    GW = 4               # q/k transposes batched per psum bank
    PG = 4               # p transposes per group
    NHEAD = 3            # pT groups pre-emitted before qw at block edge

    const = ctx.enter_context(tc.tile_pool(name="const", bufs=1))
    ident = const.tile([P, P], bf16)
    make_identity(nc, ident[:])
    ident32 = const.tile([P, P], f32, name="ident32")
    make_identity(nc, ident32[:])
    identr_t = const.tile([P, P], f32r, name="identr_t")
    nc.vector.tensor_copy(identr_t[:], ident32[:])
    ident_r = identr_t[:]

    big = ctx.enter_context(tc.tile_pool(name="big", bufs=1))
    knp = ctx.enter_context(tc.tile_pool(name="knp", bufs=KC))
    kio = ctx.enter_context(tc.tile_pool(name="kio", bufs=2))
    qio = ctx.enter_context(tc.tile_pool(name="qio", bufs=3))
    spp = ctx.enter_context(tc.tile_pool(name="spp", bufs=3))
    ptp = ctx.enter_context(tc.tile_pool(name="ptp", bufs=3))
    mio = ctx.enter_context(tc.tile_pool(name="mio", bufs=3))
    oio = ctx.enter_context(tc.tile_pool(name="oio", bufs=2))
    small = ctx.enter_context(tc.tile_pool(name="small", bufs=3))
    psum = ctx.enter_context(tc.tile_pool(name="psum", bufs=4, space="PSUM"))
    psum_t = ctx.enter_context(tc.tile_pool(name="psum_t", bufs=2, space="PSUM"))
    psum_o = ctx.enter_context(tc.tile_pool(name="psum_o", bufs=1, space="PSUM"))

    # W: DMA'd once per core, straight into the f32r tile (f32r is 4-byte
    # fp32 storage; truncation to fp22 happens at PE read time).
    wH = big.tile([P, EC * E], f32r, tag="wH")

    def emit_w_load():
        for ec in range(EC):
            nc.scalar.dma_start(wH[:, ec * E:(ec + 1) * E],
                                w_ap[ec * P:(ec + 1) * P, :])

    # deferred transpose+PV emission state: (b, row0, sp, rz, kn)
    pending = []

    def pv_transpose_group(sp, g):
        # sp is a list of per-KB-block tiles: group g reads only block g,
        # so its PV can start as soon as that block's exp+mask are done
        pt = psum_t.tile([P, PG * P], bf16, tag="tp", name="pt")
        for j in range(PG):
            nc.tensor.transpose(pt[:, j * P:(j + 1) * P],
                                sp[g][:, j * P:(j + 1) * P], ident[:])
        ptsg = ptp.tile([P, PG * P], bf16, tag="pt", name="ptsg")
        if g % 2 == 0:
            nc.scalar.copy(ptsg[:], pt[:])
        else:
            nc.vector.tensor_copy(ptsg[:], pt[:])
        return ptsg

    def emit_pv_head(st):
        return [pv_transpose_group(st[2], g)
                for g in range(min(NHEAD, KC // PG))]

    def emit_pv_tail(st, grp):
        b, row0, sp, rz, kn = st
        po = [psum_o.tile([P, EB], f32, tag=f"po{eh}", name=f"po{eh}")
              for eh in range(neb)]
        ngrp = KC // PG

        def pv_mms(g, last):
            ptsg = grp[g]
            for j in range(PG):
                kc = g * PG + j
                for eh in range(neb):
                    nc.tensor.matmul(
                        po[eh][:], ptsg[:, j * P:(j + 1) * P],
                        kn[kc][:, eh * EB:(eh + 1) * EB],
                        start=(kc == 0), stop=(last and j == PG - 1))

        for g in range(len(grp) - 1):
            pv_mms(g, last=False)
        for g in range(len(grp), ngrp):
            grp.append(pv_transpose_group(sp, g))
            pv_mms(g - 1, last=False)
        pv_mms(ngrp - 1, last=True)
        for eh in range(neb):
            ot = oio.tile([P, EB], f32, tag="ot", name="ot")
            nc.scalar.activation(ot[:], po[eh][:], AF.Copy, scale=rz[:])
            nc.sync.dma_start(
                out_ap[b, row0: row0 + P, eh * EB:(eh + 1) * EB], ot[:])

    def emit_pv(st):
        emit_pv_tail(st, emit_pv_head(st))

    def emit_block_qT(b, qb):
        q0 = qb * QB
        qTh = big.tile([P, EC, QB], f32r, tag="qTh", name="qTh")
        for qt in range(qt_per_b):
            qin = qio.tile([P, E], f32r, tag="qin", name="qin")
            nc.sync.dma_start(
                qin[:], q_ap[b, q0 + qt * P: q0 + (qt + 1) * P, :])
            for eg in range(EC // GW):
                pt = psum_t.tile([P, GW * P], f32r, tag="tp", name="pt")
                for j in range(GW):
                    ec = eg * GW + j
                    nc.tensor.transpose(
                        pt[:, j * P:(j + 1) * P],
                        qin[:, ec * P:(ec + 1) * P], ident_r)
                ptv = pt[:].rearrange("p (g c) -> p g c", g=GW)
                # alternate ACT/DVE so neither engine alone paces the
                # psum_t slot rotation at block boundaries
                eng = nc.scalar if (qt * (EC // GW) + eg) % 2 == 0 else nc.vector
                if eng is nc.scalar:
                    nc.scalar.copy(
                        qTh[:, eg * GW:(eg + 1) * GW, qt * P:(qt + 1) * P], ptv)
                else:
                    nc.vector.tensor_copy(
                        qTh[:, eg * GW:(eg + 1) * GW, qt * P:(qt + 1) * P], ptv)
        return qTh

    def emit_block_qw(qTh):
        qwTh = big.tile([P, FC * QB], f32r, tag="qwTh", name="qwTh")
        for fc in range(FC):
            ps = psum.tile([P, QB], f32, tag="ps", name="ps")
            for ec in range(EC):
                nc.tensor.matmul(
                    ps[:], wH[:, ec * E + fc * P: ec * E + (fc + 1) * P],
                    qTh[:, ec, :], start=(ec == 0), stop=(ec == EC - 1))
            nc.scalar.copy(qwTh[:, fc * QB:(fc + 1) * QB], ps[:])
        return qwTh

    def emit_k_phase(b, lo=0, hi=None, kTh=None, kn=None):
        if kTh is None:
            kTh = big.tile([P, EC, KL], f32r, tag="kTh", name="kTh")
            kn = []
        if hi is None:
            hi = KC
        for kc in range(lo, hi):
            kin = kio.tile([P, E], f32r, tag="kin", name="kin")
            nc.sync.dma_start(kin[:], k_ap[b, kc * P:(kc + 1) * P, :])
            for eg in range(EC // GW):
                pt = psum_t.tile([P, GW * P], f32r, tag="tp", name="pt")
                for j in range(GW):
                    ec = eg * GW + j
                    nc.tensor.transpose(
                        pt[:, j * P:(j + 1) * P],
                        kin[:, ec * P:(ec + 1) * P], ident_r)
                ptv = pt[:].rearrange("p (g c) -> p g c", g=GW)
                nc.scalar.copy(
                    kTh[:, eg * GW:(eg + 1) * GW, kc * P:(kc + 1) * P], ptv)
            knt = knp.tile([P, E], bf16, tag="kn", name="knt")
            nc.vector.tensor_copy(knt[:], kin[:])
            kn.append(knt)
        return kTh, kn

    def emit_scores_softmax(b, qb, qt, qwTh, kTh, kn):
        q0 = qb * QB
        sp = [spp.tile([P, KB], bf16, tag=f"sp{kb}", name="sp")
              for kb in range(nkb)]
        pm = small.tile([P, nkb], f32, tag="pm", name="pm")
        ps_list = []
        for kb in range(nkb):
            ps_s = psum.tile([P, KB], f32, tag="ps", name="ps_s")
            for fc in range(FC):
                nc.tensor.matmul(
                    ps_s[:], qwTh[:, fc * QB + qt * P: fc * QB + (qt + 1) * P],
                    kTh[:, fc, kb * KB:(kb + 1) * KB],
                    start=(fc == 0), stop=(fc == FC - 1))
            nc.vector.tensor_reduce(pm[:, kb:kb + 1], ps_s[:], axis=AX.X,
                                    op=ALU.max)
            ps_list.append(ps_s)
        negm = small.tile([P, 1], f32, tag="negm", name="negm")
        nc.vector.tensor_reduce(negm[:], pm[:], axis=AX.X, op=ALU.max,
                                negate=True)
        zp = small.tile([P, nkb], f32, tag="zp", name="zp")
        rz = small.tile([P, 1], f32, tag="rz", name="rz")
        for kb in range(nkb):
            nc.scalar.activation(sp[kb][:], ps_list[kb][:],
                                 AF.Exp, bias=negm[:])
            # z partial on DVE (unmasked sum, as in the reference)
            nc.vector.tensor_reduce(zp[:, kb:kb + 1], sp[kb][:],
                                    axis=AX.X, op=ALU.add)
            # multiplicative mask (applied after softmax numerator)
            mt = mio.tile([P, KB], i32, tag="mask", name="mt")
            nc.gpsimd.dma_start(
                mt[:], mask_ap[b, q0 + qt * P: q0 + (qt + 1) * P,
                               kb * KB:(kb + 1) * KB])
            nc.vector.scalar_tensor_tensor(
                out=sp[kb][:], in0=mt[:], scalar=1.0, in1=sp[kb][:],
                op0=ALU.mult, op1=ALU.mult)
        z = small.tile([P, 1], f32, tag="z", name="z")
        nc.vector.tensor_reduce(z[:], zp[:], axis=AX.X, op=ALU.add)
        nc.vector.reciprocal(rz[:], z[:])
        return (b, q0 + qt * P, sp, rz, kn)

    for b in range(Bl):
        qTh = emit_block_qT(b, 0)
        if b == 0:
            emit_w_load()
        # flush the deferred PV of the previous batch before its kn chunk
        # slots are rewritten by this batch's k DMAs (no-op for b==0)
        while pending:
            emit_pv(pending.pop(0))
        # split the k phase around qw: the first chunks' transposes run
        # while W/later-k DMAs are in flight; the rest after qw, by which
        # time their DMAs have landed (batch 0 waits on the 4MB W DMA, so
        # give it more pre-qw transpose work)
        head_kc = 8 if b == 0 else 6
        kTh, kn = emit_k_phase(b, lo=0, hi=head_kc)
        qwTh = emit_block_qw(qTh)
        emit_k_phase(b, lo=head_kc, kTh=kTh, kn=kn)

        for qb in range(nqb):
            head_grp = None
            if qb > 0:
                qTh = emit_block_qT(b, qb)
                # two tiles are pending at a block edge: the older one's
                # softmax is long done, so its PV fills the PE while the
                # new block's q DMAs + transpose copies land
                if len(pending) > 1:
                    emit_pv(pending.pop(0))
                if pending:
                    head_grp = emit_pv_head(pending[0])
                qwTh = emit_block_qw(qTh)

            for qt in range(qt_per_b):
                if qt == 0:
                    if pending and head_grp is not None:
                        emit_pv_tail(pending.pop(0), head_grp)
                    while pending:
                        emit_pv(pending.pop(0))
                st = emit_scores_softmax(b, qb, qt, qwTh, kTh, kn)
                pending.append(st)
                depth = 2 if qt == qt_per_b - 1 else 1
                if len(pending) > depth:
                    emit_pv(pending.pop(0))

    while pending:
        emit_pv(pending.pop(0))


def build_program(Bl, Q, KL, E, QB=512):
    nc = bacc.Bacc("TRN2", target_bir_lowering=False, debug=False)
    f32r = dt.float32r
    k_t = nc.dram_tensor("k", [Bl, KL, E], f32r, kind="ExternalInput")
    q_t = nc.dram_tensor("q", [Bl, Q, E], f32r, kind="ExternalInput")
    w_t = nc.dram_tensor("W", [E, E], f32r, kind="ExternalInput")
    m_t = nc.dram_tensor("mask", [Bl, Q, KL], dt.int32, kind="ExternalInput")
    o_t = nc.dram_tensor("out", [Bl, Q, E], dt.float32, kind="ExternalOutput")
    with tile.TileContext(nc) as tc:
        with ExitStack() as ctx:
            emit_attention(ctx, tc, k_t.ap(), q_t.ap(), w_t.ap(), m_t.ap(),
                           o_t.ap(), Bl, Q, KL, E, QB=QB)
    nc.compile()
    return nc


def kernel(k: np.ndarray, q: np.ndarray, W: np.ndarray, mask: np.ndarray,
           **run_kwargs) -> np.ndarray:
    assert k.shape == (B, K_LEN, EMB) and q.shape == (B, Q_LEN, EMB)
    assert W.shape == (EMB, EMB) and mask.shape == (B, Q_LEN, K_LEN)
    Bl = B // N_CORES
    nc = build_program(Bl, Q_LEN, K_LEN, EMB)
    in_maps = []
    for c in range(N_CORES):
        sl = slice(c * Bl, (c + 1) * Bl)
        in_maps.append({
            "k": np.ascontiguousarray(k[sl], dtype=np.float32),
            "q": np.ascontiguousarray(q[sl], dtype=np.float32),
            "W": np.ascontiguousarray(W, dtype=np.float32),
            "mask": np.ascontiguousarray(mask[sl], dtype=np.int32),
        })
    res = run_bass_kernel_spmd(nc, in_maps, core_ids=list(range(N_CORES)),
                               **run_kwargs)
    out = np.concatenate([r["out"] for r in res.results], axis=0)
    if run_kwargs.get("trace"):
        kernel.last_exec_time_ns = res.exec_time_ns
    return out


kernel.last_exec_time_ns = None
